# revision 33
# baseline (speedup 1.0000x reference)
"""Trainium2 Bass kernel for nn_Min_interval (subset min-interval selection).

Problem: for each batch row, for every subset S of the 16 input columns with
|S| <= 3, output the (xl, xu) interval of the column in S minimizing the
score s = 0.5*xl + 0.5*xu (ties broken by b = 0.2*xl + 0.8*xu, then by DP
fold order).  Output columns are sorted by subset bitmask -> [B, 696] x 2.

Device algorithm (per core, pure data-parallel over rows):
  *  The winner of any subset is argmin over its columns of s~ = xl + xu
     (exact-halving equivalence with the alpha score).  Each column is packed
     into ONE integer-valued fp32 key K = 1024*S + L with
        S = rint(8191 * (xl + xu))   (14-bit quantized score)
        L = rint(1023 * xl)          (10-bit quantized payload)
     K < 2^24, so fp32 `min` performs an exact lexicographic (score,
     payload) selection.  Every select in the subset DP collapses to a
     single `tensor_tensor min` -- 1 Vector-engine element per output
     column instead of compare + 2-plane predicated copy (3 elements).
  *  Same DP structure as before: M2 staging = the 136 |S|<=2 subsets in
     bitmask order built via 16 prefix-vs-broadcast mins; output block t =
     min(M2 prefix of width c2(t)+1, broadcast single t); a +inf sentinel
     column makes the leading "single" element fall out of the same op.
  *  The packed winners are split on-chip into two int16 planes:
        hi = convert_int16(K * 2^-10)        (Activation engine)
        lo = K - 1024*hi                     (Pool engine, fused STT)
     and DMA'd out as int16 -- HALF the HBM write traffic of fp32 planes.
     The int16 convert's rounding mode only shifts hi by delta in {0,1};
     lo lands in [-1024, 1023] and its sign recovers delta exactly on host.
  *  Host unpacks S = hi - (lo<0), L = lo + 1024*(lo<0), and reconstructs
     l = L/1023, u = S/8191 - L/1023 (quantization error ~5e-4, far inside
     the 2e-2 relative-error budget).
  *  Rows where any two columns' quantized scores differ by <= 1 (the only
     rows where the quantized argmin can disagree with the exact fp32
     reference compare, ~3% of rows) are recomputed exactly on host with
     reference semantics.  The device's OWN S values are read back from the
     singleton output columns (block offsets BOFF[t]), so the patch
     criterion is immune to device rounding-mode details.

Sharding: batch 65536 rows -> 8 cores x 8192 rows (data parallel, no comms).
Engine budget per 2048-row chunk: DVE ~14.4us (all mins), Act ~10us (S/L
quantize chains + hi extract), Pool ~16us (lo extract), DMA ~18us (22.8MB
out per core) -- DMA-bound by design.
"""

import os
import sys
import numpy as np

for _p in ("/opt/trn_rl_repo",):
    if _p not in sys.path and os.path.isdir(_p):
        sys.path.insert(0, _p)

# ----------------------------------------------------------------------------
# Problem constants (hardcoded per contest rules)
# ----------------------------------------------------------------------------
N = 16                 # input feature columns
ADD = 3                # max subset order
ALPHA = 0.5
BETA = 0.8
BATCH = 65536
N_CORES = 8
ROWS_PER_CORE = BATCH // N_CORES        # 8192
P = 128                                 # SBUF partitions
OUT_COLS = 696                          # subsets with 1<=|S|<=3 of 16
NB_DEFAULT = 16                         # rowblocks per chunk

# key packing scales
S_SCALE = 8191.0      # score quantization: S = rint(S_SCALE * (l+u)) <= 16382
L_SCALE = 1023.0      # payload quantization: L = rint(L_SCALE * l) <= 1023
MAGIC = float(2 ** 23)  # fp32 round-to-nearest-integer magic constant
SENTINEL = float(2 ** 24)  # larger than any K = 1024*S + L <= 16776191

# triangular prefix counts: c2[t] = #subsets |S|<=2 with bitmask < 2^t
C2 = [t * (t + 1) // 2 for t in range(N + 1)]
# output block offsets: block t (top bit t) starts at BOFF[t], width 1+c2[t]
BOFF = [0] * (N + 1)
for _t in range(N):
    BOFF[_t + 1] = BOFF[_t] + 1 + C2[_t]
assert BOFF[N] == OUT_COLS

M2_COLS = C2[N]  # 136 = number of |S|<=2 subsets (120 pairs + 16 singles)

C_KQ = N + 1             # 17: sentinel + 16 packed key cols
C_M2 = 1 + M2_COLS       # 137: sentinel + M2 packed keys


# ----------------------------------------------------------------------------
# Bass program builder
# ----------------------------------------------------------------------------
def build_program(rows=ROWS_PER_CORE, nb=NB_DEFAULT, reps=1):
    """Build the per-core Bass program. rows must be divisible by 128*nb.

    reps repeats the whole computation in-program (benchmarking only).
    """
    from contextlib import ExitStack
    from concourse import bacc, mybir, tile

    f32 = mybir.dt.float32
    mn = mybir.AluOpType.min
    mult = mybir.AluOpType.mult
    add = mybir.AluOpType.add
    Copy = mybir.ActivationFunctionType.Copy

    total_nb = rows // P
    assert total_nb * P == rows

    # chunk plan: tapered first and last chunks shrink the pipeline fill
    # (time to the first output store) and drain (last store after compute)
    if total_nb == 64 and nb == 16:
        plan = [14, 12, 10, 10, 8, 6, 4]
    else:
        half = max(1, nb // 2)
        if total_nb > 2 * half:
            mid = total_nb - 2 * half
            plan = [half] + [nb] * (mid // nb)
            if mid % nb:
                plan.append(mid % nb)
            plan.append(half)
        else:
            plan = [min(nb, total_nb)] * (total_nb // min(nb, total_nb))
    assert sum(plan) == total_nb
    row_off = [0]
    for nbi in plan:
        row_off.append(row_off[-1] + P * nbi)

    nc = bacc.Bacc()
    xl_d = nc.declare_dram_parameter("xl", [rows, N], f32, isOutput=False)
    xu_d = nc.declare_dram_parameter("xu", [rows, N], f32, isOutput=False)
    ok_d = nc.declare_dram_parameter("out_k", [rows, OUT_COLS], f32, isOutput=True)

    # per-chunk DRAM views: (partition, rowblock, col).  The (p nb) row
    # mapping makes each partition's rowblocks contiguous in HBM, so DMA
    # descriptors merge into nb-row runs (1KB loads, 44KB stores).
    def dram_views(ch):
        r0, r1 = row_off[ch], row_off[ch + 1]
        nbi = plan[ch]
        return (
            xl_d[:][r0:r1].rearrange("(p nb) t -> p nb t", p=P),
            xu_d[:][r0:r1].rearrange("(p nb) t -> p nb t", p=P),
            ok_d[:][r0:r1].rearrange("(p nb) o -> p nb o", p=P),
            nbi,
        )

    with ExitStack() as ctx:
        tc = ctx.enter_context(tile.TileContext(nc))
        inp = ctx.enter_context(tc.tile_pool(name="inp", bufs=2))
        scp = ctx.enter_context(tc.tile_pool(name="scp", bufs=2))
        kp = ctx.enter_context(tc.tile_pool(name="kp", bufs=2))
        m2p = ctx.enter_context(tc.tile_pool(name="m2p", bufs=2))
        okp = ctx.enter_context(tc.tile_pool(name="okp", bufs=3))

        iters = [(rep, ch) for rep in range(reps) for ch in range(len(plan))]
        in_tiles = {}

        def issue_in(i):
            # software-pipelined input loads: issued one chunk ahead on
            # the (otherwise idle) Act DMA queue, keeping the SP queue
            # store-only so stores stream back-to-back
            _, ch_i = iters[i]
            xl_v, xu_v, _, nb_i = dram_views(ch_i)
            inb = inp.tile([P, nb_i * 2 * N], f32, tag="inb")
            in3 = inb[:].rearrange("p (v nb t) -> p v nb t", v=2, t=N)
            nc.scalar.dma_start(out=in3[:, 0], in_=xl_v)
            nc.scalar.dma_start(out=in3[:, 1], in_=xu_v)
            in_tiles[i] = in3

        issue_in(0)
        for it, (_rep, ch) in enumerate(iters):
            if it + 1 < len(iters):
                issue_in(it + 1)
            _, _, ok_v, nb = dram_views(ch)
            in3 = in_tiles.pop(it)
            inl, inu = in3[:, 0], in3[:, 1]

            # scratch planes for the quantize chains
            scr = scp.tile([P, nb * 3 * N], f32, tag="scr")
            sc3 = scr[:].rearrange("p (v nb t) -> p v nb t", v=3, t=N)
            sA, sB, sC = sc3[:, 0], sc3[:, 1], sc3[:, 2]

            # L = rint(1023*l) via the fp32 round-to-integer magic constant:
            # act(l*1023 + 2^23) materializes rint(l*1023)+2^23 at the fp32
            # output write; subtracting 2^23 recovers the integer.  (Act;
            # L first: it only needs inl, which DMAs in before inu)
            nc.scalar.activation(sB, inl, Copy, bias=MAGIC, scale=L_SCALE)
            nc.scalar.activation(sC, sB, Copy, bias=-MAGIC)
            # s~ = l + u (DVE), then S = rint(4079*s~) likewise (Act)
            nc.vector.tensor_tensor(sA, inl, inu, add)
            nc.scalar.activation(sB, sA, Copy, bias=MAGIC, scale=S_SCALE)
            nc.scalar.activation(sA, sB, Copy, bias=-MAGIC)

            # packed keys: K = S*1024 + L  -> Kp cols 1..16, sentinel col 0
            kt = kp.tile([P, nb * C_KQ], f32, tag="kt")
            k3 = kt[:].rearrange("p (nb q) -> p nb q", q=C_KQ)
            nc.gpsimd.memset(k3[:, :, 0:1], SENTINEL)
            nc.vector.scalar_tensor_tensor(
                k3[:, :, 1:1 + N], sA, 1024.0, sC, mult, add)

            # ---------------- pairs stage: fill M2 with packed mins ---------
            m2 = m2p.tile([P, nb * C_M2], f32, tag="m2")
            m3 = m2[:].rearrange("p (nb q) -> p nb q", q=C_M2)
            nc.gpsimd.memset(m3[:, :, 0:1], SENTINEL)
            for j in range(N):
                W = j + 1
                q0 = 1 + C2[j]
                ls = k3[:, :, 0:W]
                rs = k3[:, :, 1 + j:2 + j].to_broadcast((P, nb, W))
                nc.vector.tensor_tensor(m3[:, :, q0:q0 + W], ls, rs, mn)

            # ---------------- final stage: output blocks as packed mins -----
            # o3 is the output staging itself: the packed fp32 K winners are
            # DMA'd out raw (integer-valued fp32) and split on the host --
            # zero on-chip extraction work
            ok = okp.tile([P, nb * OUT_COLS], f32, tag="ok")
            o3 = ok[:].rearrange("p (nb o) -> p nb o", o=OUT_COLS)

            for t in range(N):
                W = C2[t] + 1
                b0 = BOFF[t]
                ls = m3[:, :, 0:W]
                rs = k3[:, :, 1 + t:2 + t].to_broadcast((P, nb, W))
                nc.vector.tensor_tensor(o3[:, :, b0:b0 + W], ls, rs, mn)

            # single store per chunk: (p nb) row mapping merges descriptors
            # into one nb*2784-byte run per partition
            nc.sync.dma_start(out=ok_v, in_=o3)

    nc.finalize()
    return nc


# ----------------------------------------------------------------------------
# Exact reference semantics in numpy (for quantization-ambiguous rows)
# ----------------------------------------------------------------------------
def _build_plan():
    from itertools import combinations

    items = list(range(N))
    index_dict = {(i,): i for i in items}
    count = N
    plan = []
    for length in range(2, min(ADD, N) + 1):
        combos = list(combinations(items, length))
        left = np.array([index_dict[c[1:]] for c in combos], dtype=np.int32)
        right = np.array([index_dict[c[:-1]] for c in combos], dtype=np.int32)
        for c in combos:
            index_dict[c] = count
            count += 1
        plan.append((left, right))

    def bitmask(c):
        m = 0
        for i in c:
            m |= 1 << i
        return m

    order = np.array(
        [index_dict[c] for c in sorted(index_dict, key=bitmask)], dtype=np.int32
    )
    return plan, order


_PLAN_CACHE = None


def _reference_numpy(xl, xu):
    """Bit-exact fp32 reproduction of the jax reference for given rows."""
    global _PLAN_CACHE
    if _PLAN_CACHE is None:
        _PLAN_CACHE = _build_plan()
    plan, order = _PLAN_CACHE
    a0 = np.float32(1.0 - ALPHA)
    a1 = np.float32(ALPHA)
    b0 = np.float32(1.0 - BETA)
    b1 = np.float32(BETA)
    mat_l, mat_u = xl.astype(np.float32), xu.astype(np.float32)
    for left_idx, right_idx in plan:
        ll, lu = mat_l[:, left_idx], mat_u[:, left_idx]
        rl, ru = mat_l[:, right_idx], mat_u[:, right_idx]
        cur = a0 * ll + a1 * lu
        nxt = a0 * rl + a1 * ru
        bcur = b0 * ll + b1 * lu
        bnxt = b0 * rl + b1 * ru
        choose_right = np.where(cur == nxt, bcur > bnxt, cur > nxt)
        res_l = np.where(choose_right, rl, ll)
        res_u = np.where(choose_right, ru, lu)
        mat_l = np.concatenate([mat_l, res_l], axis=1)
        mat_u = np.concatenate([mat_u, res_u], axis=1)
    return mat_l[:, order], mat_u[:, order]


# ----------------------------------------------------------------------------
# Host entry point
# ----------------------------------------------------------------------------
_PROGRAM_CACHE = {}


def _get_program(rows, nb):
    key = (rows, nb)
    if key not in _PROGRAM_CACHE:
        _PROGRAM_CACHE[key] = build_program(rows, nb)
    return _PROGRAM_CACHE[key]


def _reconstruct(k):
    """Unpack the device's packed fp32 K plane into fp32 (out_l, out_u, S)."""
    K = k.astype(np.int32)
    L = (K & 1023).astype(np.float32)
    S = (K >> 10).astype(np.float32)
    out_l = L * np.float32(1.0 / L_SCALE)
    out_u = S * np.float32(1.0 / S_SCALE) - L * np.float32(1.0 / L_SCALE)
    return out_l, out_u, S


def kernel(xl, xu):
    from concourse.bass_utils import run_bass_kernel_spmd

    xl = np.ascontiguousarray(np.asarray(xl), dtype=np.float32)
    xu = np.ascontiguousarray(np.asarray(xu), dtype=np.float32)
    assert xl.shape == (BATCH, N) and xu.shape == (BATCH, N)

    nc = _get_program(ROWS_PER_CORE, NB_DEFAULT)

    in_maps = []
    for c in range(N_CORES):
        sl = slice(c * ROWS_PER_CORE, (c + 1) * ROWS_PER_CORE)
        in_maps.append({"xl": xl[sl], "xu": xu[sl]})

    res = run_bass_kernel_spmd(nc, in_maps, list(range(N_CORES))).results

    k = np.concatenate([r["out_k"] for r in res], axis=0)
    out_l, out_u, S = _reconstruct(k)

    # Patch rows where the quantized score of any two columns is within 1:
    # only there can the packed argmin disagree with the exact reference
    # compare (score inversion or tie).  S values are the DEVICE's own,
    # read back from the singleton output columns.
    s_single = S[:, np.array(BOFF[:N], dtype=np.int64)]
    ss = np.sort(s_single, axis=1)
    bad = (np.diff(ss, axis=1) <= 1.0).any(axis=1)
    rows = np.nonzero(bad)[0]
    if rows.size:
        pl, pu = _reference_numpy(xl[rows], xu[rows])
        out_l[rows] = pl
        out_u[rows] = pu

    return out_l, out_u


# revision 35
# speedup vs baseline: 34.9991x; 34.9991x over previous
"""Trainium2 Bass kernel for nn_Min_interval — v3: u16 argmin-index keys.

Device packs each column into a uint16 key K = 16*S + col_idx with
S = rint(2047*(xl+xu)) (12-bit quantized score).  All subset selects are
2-byte `tensor_tensor min` ops on the Vector engine, which hit the DVE
2x perf mode (staging is column-major [P, cols, rowblocks] so every
operand's innermost dim is packed; the per-block broadcast rides the
middle dim).  The winner's column INDEX comes back in the low 4 bits;
the host gathers the exact fp32 (xl, xu) values by index, so unpatched
rows are bit-exact.  Rows where any two quantized scores differ by <= 1
(~12%) are recomputed exactly on host.  Output traffic: one u16 plane,
1392 B/row (4x less than fp32 l/u planes).
"""

import os
import sys
import numpy as np

for _p in ("/opt/trn_rl_repo",):
    if _p not in sys.path and os.path.isdir(_p):
        sys.path.insert(0, _p)

N = 16
ADD = 3
ALPHA = 0.5
BETA = 0.8
BATCH = 65536
N_CORES = 8
ROWS_PER_CORE = BATCH // N_CORES        # 8192
P = 128
OUT_COLS = 696
NB_DEFAULT = 16

S_SCALE = 2047.0        # S = rint(S_SCALE*(l+u)) <= 4094 (12 bits)
MAGIC = float(2 ** 23)
SENTINEL_U16 = 65535    # > any K = 16*S + idx <= 65519

C2 = [t * (t + 1) // 2 for t in range(N + 1)]
BOFF = [0] * (N + 1)
for _t in range(N):
    BOFF[_t + 1] = BOFF[_t] + 1 + C2[_t]
assert BOFF[N] == OUT_COLS

C_KQ = N + 1
C_M2 = 1 + C2[N]


def _chunk_plan(total_nb, nb):
    if total_nb == 64 and nb == 16:
        return [16, 14, 12, 10, 8, 4]
    half = max(1, nb // 2)
    if total_nb > 2 * half:
        mid = total_nb - 2 * half
        plan = [half] + [nb] * (mid // nb)
        if mid % nb:
            plan.append(mid % nb)
        plan.append(half)
        return plan
    m = min(nb, total_nb)
    return [m] * (total_nb // m)


def build_program(rows=ROWS_PER_CORE, nb=NB_DEFAULT, reps=1):
    from contextlib import ExitStack
    from concourse import bacc, mybir, tile

    f32 = mybir.dt.float32
    u16 = mybir.dt.uint16
    mn = mybir.AluOpType.min
    mult = mybir.AluOpType.mult
    add = mybir.AluOpType.add
    Copy = mybir.ActivationFunctionType.Copy

    total_nb = rows // P
    assert total_nb * P == rows
    plan = _chunk_plan(total_nb, nb)
    assert sum(plan) == total_nb
    nb_max = max(plan)
    row_off = [0]
    for nbi in plan:
        row_off.append(row_off[-1] + P * nbi)

    nc = bacc.Bacc()
    xl_d = nc.declare_dram_parameter("xl", [rows, N], f32, isOutput=False)
    xu_d = nc.declare_dram_parameter("xu", [rows, N], f32, isOutput=False)
    # flat output: per chunk a [P, OUT_COLS, nb] column-major slab; the host
    # transposes back.  Contiguous 2*OUT_COLS*nb-byte run per partition.
    ok_d = nc.declare_dram_parameter(
        "out_k", [rows * OUT_COLS], u16, isOutput=True)

    def dram_views(ch):
        r0, r1 = row_off[ch], row_off[ch + 1]
        nbi = plan[ch]
        return (
            xl_d[:][r0:r1].rearrange("(nb p) t -> p nb t", p=P),
            xu_d[:][r0:r1].rearrange("(nb p) t -> p nb t", p=P),
            ok_d[:][r0 * OUT_COLS:r1 * OUT_COLS].rearrange(
                "(p x) -> p x", p=P),
            nbi,
        )

    with ExitStack() as ctx:
        tc = ctx.enter_context(tile.TileContext(nc))
        cst = ctx.enter_context(tc.tile_pool(name="cst", bufs=1))
        inp = ctx.enter_context(tc.tile_pool(name="inp", bufs=2))
        scp = ctx.enter_context(tc.tile_pool(name="scp", bufs=2))
        kp = ctx.enter_context(tc.tile_pool(name="kp", bufs=2))
        m2p = ctx.enter_context(tc.tile_pool(name="m2p", bufs=2))
        okp = ctx.enter_context(tc.tile_pool(name="okp", bufs=3))

        # one-time column-index plane: row t holds float(t)
        idxt = cst.tile([P, N * nb_max], f32, tag="idx")
        idx3 = idxt[:].rearrange("p (t nb) -> p t nb", t=N)
        for t in range(N):
            nc.gpsimd.memset(idx3[:, t:t + 1, :], float(t))

        iters = [(rep, ch) for rep in range(reps) for ch in range(len(plan))]
        in_tiles = {}

        def issue_in(i):
            _, ch_i = iters[i]
            xl_v, xu_v, _, nb_i = dram_views(ch_i)
            inb = inp.tile([P, nb_i * 2 * N], f32, tag="inb")
            in3 = inb[:].rearrange("p (v nb t) -> p v nb t", v=2, t=N)
            nc.sync.dma_start(out=in3[:, 0], in_=xl_v)
            nc.sync.dma_start(out=in3[:, 1], in_=xu_v)
            in_tiles[i] = in3

        issue_in(0)
        for it, (_rep, ch) in enumerate(iters):
            if it + 1 < len(iters):
                issue_in(it + 1)
            _, _, ok_v, nb = dram_views(ch)
            in3 = in_tiles.pop(it)

            # s~ = l + u (row-major), then transpose to column-major and
            # quantize: S = rint(2047*s~) via the fp32 magic constant
            scr = scp.tile([P, nb * 2 * N], f32, tag="scr")
            sc3 = scr[:].rearrange("p (v t nb) -> p v t nb", v=2, t=N)
            sR, sT = sc3[:, 0], sc3[:, 1]
            # writing the s-add through a transposed view lands the data
            # column-major directly -- no separate transpose pass
            nc.vector.tensor_tensor(
                sR.rearrange("p t nb -> p nb t"), in3[:, 0], in3[:, 1], add)
            nc.scalar.activation(sT, sR, Copy, bias=MAGIC, scale=S_SCALE)
            nc.scalar.activation(sR, sT, Copy, bias=-MAGIC)

            # keys: K = S*16 + t -> kt rows 1..16, sentinel row 0
            kt = kp.tile([P, C_KQ * nb], u16, tag="kt")
            k3 = kt[:].rearrange("p (q nb) -> p q nb", q=C_KQ)
            nc.gpsimd.memset(k3[:, 0:1, :], SENTINEL_U16)
            nc.vector.scalar_tensor_tensor(
                k3[:, 1:1 + N, :], sR, 16.0, idx3[:, :, :nb], mult, add)

            # pairs: M2 block j = min(prefix, broadcast single j)
            m2 = m2p.tile([P, C_M2 * nb], u16, tag="m2")
            m3 = m2[:].rearrange("p (q nb) -> p q nb", q=C_M2)
            nc.gpsimd.memset(m3[:, 0:1, :], SENTINEL_U16)
            for j in range(N):
                W = j + 1
                q0 = 1 + C2[j]
                ls = k3[:, 0:W, :]
                rs = k3[:, 1 + j:2 + j, :].to_broadcast((P, W, nb))
                nc.vector.tensor_tensor(m3[:, q0:q0 + W, :], ls, rs, mn)

            # finals: block t = min(M2 prefix, broadcast single t)
            ok = okp.tile([P, OUT_COLS * nb], u16, tag="ok")
            o3 = ok[:].rearrange("p (o nb) -> p o nb", o=OUT_COLS)
            for t in range(N):
                W = C2[t] + 1
                b0 = BOFF[t]
                ls = m3[:, 0:W, :]
                rs = k3[:, 1 + t:2 + t, :].to_broadcast((P, W, nb))
                nc.vector.tensor_tensor(o3[:, b0:b0 + W, :], ls, rs, mn)

            # store in ~44-column waves alternating between the sync and
            # gpsimd DMA queues: ~128 descriptors of ~1.4KB per wave, the
            # many-moderate-descriptor pattern the HW DMA engines fan out
            o2 = ok[:]
            step = 44
            for wi, c0 in enumerate(range(0, OUT_COLS, step)):
                c1 = min(c0 + step, OUT_COLS)
                src_ap = o2[:, c0 * nb:c1 * nb]
                dst_ap = ok_v[:, c0 * nb:c1 * nb]
                if wi % 2 == 0:
                    nc.sync.dma_start(out=dst_ap, in_=src_ap)
                else:
                    nc.gpsimd.dma_start(out=dst_ap, in_=src_ap)

    nc.finalize()
    return nc


# ----------------------------------------------------------------------------
# Exact reference semantics in numpy (for quantization-ambiguous rows)
# ----------------------------------------------------------------------------
def _build_plan():
    from itertools import combinations

    items = list(range(N))
    index_dict = {(i,): i for i in items}
    count = N
    plan = []
    for length in range(2, min(ADD, N) + 1):
        combos = list(combinations(items, length))
        left = np.array([index_dict[c[1:]] for c in combos], dtype=np.int32)
        right = np.array([index_dict[c[:-1]] for c in combos], dtype=np.int32)
        for c in combos:
            index_dict[c] = count
            count += 1
        plan.append((left, right))

    def bitmask(c):
        m = 0
        for i in c:
            m |= 1 << i
        return m

    order = np.array(
        [index_dict[c] for c in sorted(index_dict, key=bitmask)], dtype=np.int32
    )
    return plan, order


_PLAN_CACHE = None


def _reference_numpy(xl, xu):
    global _PLAN_CACHE
    if _PLAN_CACHE is None:
        _PLAN_CACHE = _build_plan()
    plan, order = _PLAN_CACHE
    a0 = np.float32(1.0 - ALPHA)
    a1 = np.float32(ALPHA)
    b0 = np.float32(1.0 - BETA)
    b1 = np.float32(BETA)
    mat_l, mat_u = xl.astype(np.float32), xu.astype(np.float32)
    for left_idx, right_idx in plan:
        ll, lu = mat_l[:, left_idx], mat_u[:, left_idx]
        rl, ru = mat_l[:, right_idx], mat_u[:, right_idx]
        cur = a0 * ll + a1 * lu
        nxt = a0 * rl + a1 * ru
        bcur = b0 * ll + b1 * lu
        bnxt = b0 * rl + b1 * ru
        choose_right = np.where(cur == nxt, bcur > bnxt, cur > nxt)
        res_l = np.where(choose_right, rl, ll)
        res_u = np.where(choose_right, ru, lu)
        mat_l = np.concatenate([mat_l, res_l], axis=1)
        mat_u = np.concatenate([mat_u, res_u], axis=1)
    return mat_l[:, order], mat_u[:, order]


_PROGRAM_CACHE = {}


def _get_program(rows, nb):
    key = (rows, nb)
    if key not in _PROGRAM_CACHE:
        _PROGRAM_CACHE[key] = build_program(rows, nb)
    return _PROGRAM_CACHE[key]


def _decode_core(flat, rows, nb=NB_DEFAULT):
    """Per-core flat u16 slab -> row-major K [rows, OUT_COLS]."""
    plan = _chunk_plan(rows // P, nb)
    out = np.empty((rows, OUT_COLS), dtype=np.uint16)
    r0 = 0
    base = 0
    for nbi in plan:
        n = P * nbi * OUT_COLS
        slab = flat[base:base + n].reshape(P, OUT_COLS, nbi)
        # rows within the chunk are (nb p)-ordered
        out[r0:r0 + P * nbi] = slab.transpose(2, 0, 1).reshape(P * nbi, OUT_COLS)
        base += n
        r0 += P * nbi
    return out


def kernel(xl, xu):
    from concourse.bass_utils import run_bass_kernel_spmd

    xl = np.ascontiguousarray(np.asarray(xl), dtype=np.float32)
    xu = np.ascontiguousarray(np.asarray(xu), dtype=np.float32)
    assert xl.shape == (BATCH, N) and xu.shape == (BATCH, N)

    nc = _get_program(ROWS_PER_CORE, NB_DEFAULT)

    in_maps = []
    for c in range(N_CORES):
        sl = slice(c * ROWS_PER_CORE, (c + 1) * ROWS_PER_CORE)
        in_maps.append({"xl": xl[sl], "xu": xu[sl]})

    res = run_bass_kernel_spmd(nc, in_maps, list(range(N_CORES))).results

    K = np.concatenate(
        [_decode_core(r["out_k"], ROWS_PER_CORE) for r in res], axis=0)
    Ki = K.astype(np.int32)
    S = (Ki >> 4).astype(np.float32)
    idx = (Ki & 15).astype(np.int64)

    # winner values gathered EXACTLY from the original inputs
    out_l = np.take_along_axis(xl, idx, axis=1)
    out_u = np.take_along_axis(xu, idx, axis=1)

    # patch rows where any two quantized scores are within 1 (the only rows
    # where the quantized argmin can disagree with the reference compare)
    s_single = S[:, np.array(BOFF[:N], dtype=np.int64)]
    ss = np.sort(s_single, axis=1)
    bad = (np.diff(ss, axis=1) <= 1.0).any(axis=1)
    rows = np.nonzero(bad)[0]
    if rows.size:
        pl, pu = _reference_numpy(xl[rows], xu[rows])
        out_l[rows] = pl
        out_u[rows] = pu

    return out_l, out_u
